# revision 10
# baseline (speedup 1.0000x reference)
"""Trainium2 Bass kernel for nn_CrossAttention (per-head-LN cross attention).

Sharding: 16 heads / 8 cores -> 2 heads per core, both batch elements on every
core (attention is embarrassingly parallel over (B, H)). Each core computes its
128 output channels [128p, 128p+128) of the final [S, B, 1024] output.

Device algorithm (per core, all matmuls bf16 with f32 PSUM accumulation):
  - Projections computed transposed: Y^T[o, t] (o = core's 128 channels,
    t = b*S + s), via stationary W^T chunks against streamed X^T tiles.
  - Per-head LayerNorm with matmul-broadcast stats: mu_bc = blockones.T @ Y
    (f32), var_bc = blockones.T @ (Y - mu)^2 (bf16), rstd via ACT Sqrt + DVE
    reciprocal. The 1/sqrt(head) score scale is folded into Q's Sqrt scale.
  - V transposed back to natural [k, d] per (b,h) via PE transposes, with a
    ones column appended (row 64 of the PV output then holds the softmax
    denominator).
  - scores^T[k, q] = K^T.T @ Q^T per (b,h); softmax without max subtraction
    (scores are O(1) after LN; exp cannot overflow); exp on ACT directly from
    PSUM; PV: out^T[d|den, q] accumulated over k chunks; PE-transpose back to
    natural [q, d] and multiply by 1/den.
"""

import os
import numpy as np
import ml_dtypes

import concourse.bacc as bacc
import concourse.mybir as mybir
import concourse.tile as tile
from concourse.bass_utils import run_bass_kernel_spmd

F32 = mybir.dt.float32
BF16 = mybir.dt.bfloat16
AF = mybir.ActivationFunctionType
ALU = mybir.AluOpType

S = 2048
B = 2
DIM = 1024
NHEAD = 16
HEAD = 64
EPS = 1e-5
NCORES = 8
OC = DIM // NCORES          # 128 output channels per core
HPC = OC // HEAD            # 2 heads per core
T = S * B                   # 4096 tokens (t = b*S + s)
TCH = 512                   # token chunk (matmul moving free dim)
NT = T // TCH               # 8 token chunks
NCC = DIM // 128            # 8 contraction chunks

LAST_RESULT = None


def _emit(tc, aps, flags):
    from contextlib import ExitStack

    nc = tc.nc
    names = ("q", "k", "v")

    stack = ExitStack()
    consts = stack.enter_context(tc.tile_pool(name="consts", bufs=1))
    # stationary weights W^T as [128, 8, 128] (partition=c within chunk)
    wt_sb = {}
    for n in names:
        t = consts.tile([128, NCC, OC], BF16, tag=f"wt_{n}")
        nc.sync.dma_start(out=t, in_=aps[f"wt_{n}"].rearrange("(a p) m -> p a m", p=128))
        wt_sb[n] = t
    bones32 = consts.tile([128, OC], F32, tag="bones32")
    nc.sync.dma_start(out=bones32, in_=aps["blockones_f32"])
    bones16 = consts.tile([128, OC], BF16, tag="bones16")
    nc.sync.dma_start(out=bones16, in_=aps["blockones_bf16"])
    id16 = consts.tile([128, 128], BF16, tag="id16")
    nc.sync.dma_start(out=id16, in_=aps["identity_bf16"])
    id32 = consts.tile([128, 128], F32, tag="id32")
    nc.sync.dma_start(out=id32, in_=aps["identity_f32"])
    eps_q = consts.tile([128, 1], F32, tag="eps_q")
    nc.vector.memset(eps_q, float(HEAD * EPS))
    eps_kv = consts.tile([128, 1], F32, tag="eps_kv")
    nc.vector.memset(eps_kv, float(EPS))
    extra = {}
    for n in names:
        if flags[f"bias_{n}"]:
            t = consts.tile([128, 1], F32, tag=f"bcol_{n}")
            nc.sync.dma_start(out=t, in_=aps[f"bcol_{n}"])
            extra[f"bcol_{n}"] = t
        if flags[f"gb_{n}"]:
            tg = consts.tile([128, 1], F32, tag=f"gcol_{n}")
            nc.sync.dma_start(out=tg, in_=aps[f"gcol_{n}"])
            tb = consts.tile([128, 1], F32, tag=f"betacol_{n}")
            nc.sync.dma_start(out=tb, in_=aps[f"betacol_{n}"])
            extra[f"gcol_{n}"] = tg
            extra[f"betacol_{n}"] = tb

    ln_pool = stack.enter_context(tc.tile_pool(name="ln", bufs=1))
    ln_sb = {
        n: ln_pool.tile([128, T], BF16, tag=f"ln_{n}", name=f"ln_{n}") for n in names
    }

    # ---------------- phase 1: projections + per-head LN ----------------
    with (
        tc.tile_pool(name="xload", bufs=6) as xload,
        tc.tile_pool(name="p1tmp", bufs=3) as p1tmp,
        tc.tile_pool(name="ps_y", bufs=2, space="PSUM") as ps_y_pool,
        tc.tile_pool(name="ps_stat", bufs=2, space="PSUM") as ps_stat_pool,
    ):
        for tch in range(NT):
            tsl = slice(tch * TCH, (tch + 1) * TCH)
            for n in names:
                ps_y = ps_y_pool.tile([128, TCH], F32, tag="ps_y")
                for c in range(NCC):
                    xt = xload.tile([128, TCH], BF16, tag="xt")
                    nc.sync.dma_start(
                        out=xt, in_=aps[f"xt_{n}"][c * 128:(c + 1) * 128, tsl]
                    )
                    nc.tensor.matmul(
                        ps_y, lhsT=wt_sb[n][:, c, :], rhs=xt,
                        start=(c == 0), stop=(c == NCC - 1),
                    )
                yt32 = p1tmp.tile([128, TCH], F32, tag="yt32")
                if flags[f"bias_{n}"]:
                    nc.vector.tensor_scalar(
                        out=yt32, in0=ps_y, scalar1=extra[f"bcol_{n}"],
                        scalar2=None, op0=ALU.add,
                    )
                else:
                    nc.vector.tensor_copy(out=yt32, in_=ps_y)
                ps_mu = ps_stat_pool.tile([128, TCH], F32, tag="ps_mu")
                nc.tensor.matmul(ps_mu, lhsT=bones32, rhs=yt32, start=True, stop=True)
                t_c = p1tmp.tile([128, TCH], F32, tag="t_c")
                nc.vector.tensor_sub(t_c, yt32, ps_mu)
                sq = p1tmp.tile([128, TCH], BF16, tag="sq")
                nc.vector.tensor_mul(sq, t_c, t_c)
                ps_var = ps_stat_pool.tile([128, TCH], F32, tag="ps_var")
                nc.tensor.matmul(ps_var, lhsT=bones16, rhs=sq, start=True, stop=True)
                std = p1tmp.tile([128, TCH], F32, tag="std")
                if n == "q":
                    # std8 = sqrt(HEAD*var + HEAD*eps) = sqrt(HEAD)*sqrt(var+eps)
                    nc.scalar.activation(std, ps_var, AF.Sqrt, bias=eps_q, scale=float(HEAD))
                else:
                    nc.scalar.activation(std, ps_var, AF.Sqrt, bias=eps_kv, scale=1.0)
                s_t = p1tmp.tile([128, TCH], F32, tag="s_t")
                nc.vector.reciprocal(s_t, std)
                if flags[f"gb_{n}"]:
                    lnf = p1tmp.tile([128, TCH], F32, tag="lnf")
                    nc.vector.tensor_mul(lnf, t_c, s_t)
                    nc.vector.tensor_scalar(
                        out=ln_sb[n][:, tsl], in0=lnf,
                        scalar1=extra[f"gcol_{n}"], scalar2=extra[f"betacol_{n}"],
                        op0=ALU.mult, op1=ALU.add,
                    )
                else:
                    nc.vector.tensor_mul(ln_sb[n][:, tsl], t_c, s_t)

    # ---------------- phases 2+3: per-(b, head) attention ----------------
    QCH = 512
    NQ = S // QCH            # 4 q chunks per (b, h)
    NKT = S // 128           # 16 k tiles per (b, h)
    WV = 2                   # k tiles per exp wave

    with (
        tc.tile_pool(name="vnat", bufs=2) as vnat_pool,
        tc.tile_pool(name="attn", bufs=3) as attn_pool,
        tc.tile_pool(name="p3tmp", bufs=3) as p3tmp,
        tc.tile_pool(name="ostage", bufs=2) as ostage_pool,
        tc.tile_pool(name="ps_wave", bufs=2, space="PSUM") as ps_wave_pool,
        tc.tile_pool(name="ps_small", bufs=2, space="PSUM") as ps_small_pool,
    ):
        for b in range(B):
            for hl in range(HPC):
                dsl = slice(HEAD * hl, HEAD * (hl + 1))
                t0 = b * S
                # V natural [k, d | ones] tiles for this (b, h)
                vnat = vnat_pool.tile([128, NKT, HEAD + 1], BF16, tag="vnat")
                nc.vector.memset(vnat[:, :, HEAD:HEAD + 1], 1.0)
                for kt in range(NKT):
                    ps_tr = ps_small_pool.tile([128, HEAD], BF16, tag="tr")
                    nc.tensor.transpose(
                        ps_tr,
                        ln_sb["v"][dsl, t0 + kt * 128: t0 + (kt + 1) * 128],
                        id16[dsl, dsl],
                    )
                    nc.vector.tensor_copy(out=vnat[:, kt, 0:HEAD], in_=ps_tr)

                ostage = ostage_pool.tile([128, S // 128, HEAD], F32, tag="ostage")
                for qc in range(NQ):
                    qsl = slice(t0 + qc * QCH, t0 + (qc + 1) * QCH)
                    ps_o = ps_small_pool.tile([HEAD + 1, QCH], F32, tag="ps_o")
                    for wv in range(NKT // WV):
                        ps_wave = ps_wave_pool.tile([128, WV, QCH], F32, tag="wave")
                        for j in range(WV):
                            kt = wv * WV + j
                            nc.tensor.matmul(
                                ps_wave[:, j, :],
                                lhsT=ln_sb["k"][dsl, t0 + kt * 128: t0 + (kt + 1) * 128],
                                rhs=ln_sb["q"][dsl, qsl],
                                start=True, stop=True,
                            )
                        at = attn_pool.tile([128, WV, QCH], BF16, tag="at")
                        nc.scalar.activation(at, ps_wave, AF.Exp)
                        for j in range(WV):
                            kt = wv * WV + j
                            nc.tensor.matmul(
                                ps_o, lhsT=vnat[:, kt, :], rhs=at[:, j, :],
                                start=(kt == 0), stop=(kt == NKT - 1),
                            )
                    oT = p3tmp.tile([HEAD + 1, QCH], F32, tag="oT")
                    nc.vector.tensor_copy(out=oT, in_=ps_o)
                    for sub in range(QCH // 128):
                        ps_tro = ps_small_pool.tile([128, HEAD + 1], F32, tag="tr")
                        nc.tensor.transpose(
                            ps_tro, oT[:, sub * 128:(sub + 1) * 128],
                            id32[:HEAD + 1, :HEAD + 1],
                        )
                        inv = p3tmp.tile([128, 1], F32, tag="inv")
                        nc.vector.reciprocal(inv, ps_tro[:, HEAD:HEAD + 1])
                        nc.vector.tensor_scalar_mul(
                            out=ostage[:, qc * (QCH // 128) + sub, :],
                            in0=ps_tro[:, 0:HEAD], scalar1=inv,
                        )
                # store this (b, h): out[s, b, hl*64 : hl*64+64]
                dst = aps["out"][:, b, HEAD * hl: HEAD * (hl + 1)]
                dst = dst.rearrange("(n p) c -> p n c", p=128)
                nc.sync.dma_start(out=dst, in_=ostage)

    stack.close()


def _build(flags_key, flags, input_specs):
    nc = bacc.Bacc("TRN2", target_bir_lowering=False, debug=False)
    aps = {}
    for name, shape, dt in input_specs:
        aps[name] = nc.dram_tensor(name, list(shape), dt, kind="ExternalInput").ap()
    aps["out"] = nc.dram_tensor("out", [S, B, OC], F32, kind="ExternalOutput").ap()
    with tile.TileContext(nc) as tc:
        _emit(tc, aps, flags)
    nc.compile()
    return nc


_CACHE = {}


def kernel(**inputs):
    global LAST_RESULT
    bf16 = ml_dtypes.bfloat16
    f32 = np.float32

    Q, K, V = (np.asarray(inputs[n], f32) for n in ("Q", "K", "V"))
    W = {n: np.asarray(inputs["W" + n.upper()], f32) for n in ("q", "k", "v")}
    bias = {n: np.asarray(inputs["b" + n.upper()], f32) for n in ("q", "k", "v")}
    g = {n: np.asarray(inputs["g" + n.upper()], f32) for n in ("q", "k", "v")}
    beta = {n: np.asarray(inputs["beta" + n.upper()], f32) for n in ("q", "k", "v")}

    # X^T [c, t] with t = b*S + s
    xt = {
        "q": np.ascontiguousarray(Q.transpose(2, 1, 0).reshape(DIM, T)).astype(bf16),
        "k": np.ascontiguousarray(K.transpose(2, 1, 0).reshape(DIM, T)).astype(bf16),
        "v": np.ascontiguousarray(V.transpose(2, 1, 0).reshape(DIM, T)).astype(bf16),
    }
    blockones = np.kron(np.eye(2, dtype=f32), np.ones((HEAD, HEAD), f32)) / HEAD
    ident = np.eye(128, dtype=f32)

    flags = {}
    for n in ("q", "k", "v"):
        flags[f"bias_{n}"] = bool(np.any(bias[n] != 0.0))
        flags[f"gb_{n}"] = bool(np.any(g[n] != 1.0) or np.any(beta[n] != 0.0))
    flags_key = tuple(sorted(flags.items()))

    # per-core input maps
    in_maps = []
    shared = {
        "xt_q": xt["q"], "xt_k": xt["k"], "xt_v": xt["v"],
        "blockones_f32": blockones,
        "blockones_bf16": blockones.astype(bf16),
        "identity_f32": ident,
        "identity_bf16": ident.astype(bf16),
    }
    for p in range(NCORES):
        sl = slice(OC * p, OC * (p + 1))
        m = dict(shared)
        for n in ("q", "k", "v"):
            m[f"wt_{n}"] = np.ascontiguousarray(W[n][sl].T).astype(bf16)
            if flags[f"bias_{n}"]:
                m[f"bcol_{n}"] = np.ascontiguousarray(bias[n][sl]).reshape(128, 1)
            if flags[f"gb_{n}"]:
                m[f"gcol_{n}"] = np.tile(g[n], HPC).astype(f32).reshape(128, 1)
                bcol = np.tile(beta[n], HPC).astype(f32)
                if n == "q":
                    bcol = (bcol / np.sqrt(HEAD)).astype(f32)
                m[f"betacol_{n}"] = bcol.reshape(128, 1)
        in_maps.append(m)

    if flags_key not in _CACHE:
        input_specs = []
        for name, arr in in_maps[0].items():
            dt = BF16 if arr.dtype == bf16 else F32
            input_specs.append((name, arr.shape, dt))
        _CACHE[flags_key] = _build(flags_key, flags, input_specs)
    nc = _CACHE[flags_key]

    trace = bool(os.environ.get("KERNEL_TRACE"))
    tmpdir = os.environ.get("KERNEL_TRACE_DIR") or None
    res = run_bass_kernel_spmd(
        nc, in_maps, core_ids=list(range(NCORES)), trace=trace, tmpdir=tmpdir
    )
    LAST_RESULT = res
    out = np.concatenate(
        [np.asarray(res.results[p]["out"], f32) for p in range(NCORES)], axis=2
    )
    return out


# revision 18
# speedup vs baseline: 1.2666x; 1.2666x over previous
"""Trainium2 Bass kernel for nn_CrossAttention (per-head-LN cross attention).

Sharding: 16 heads / 8 cores -> 2 heads per core, both batch elements on every
core (attention is embarrassingly parallel over (B, H)). Each core computes its
128 output channels [128p, 128p+128) of the final [S, B, 1024] output.

Device algorithm (per core, all matmuls bf16 with f32 PSUM accumulation):
  - Projections computed transposed: Y^T[o, t] (o = core's 128 channels,
    t = b*S + s), via stationary W^T chunks against streamed X^T tiles.
  - Per-head LayerNorm with matmul-broadcast stats: mu_bc = blockones.T @ Y
    (f32), var_bc = blockones.T @ (Y - mu)^2 (bf16), rstd via ACT Sqrt + DVE
    reciprocal. The 1/sqrt(head) score scale is folded into Q's Sqrt scale.
  - V transposed back to natural [k, d] per (b,h) via PE transposes, with a
    ones column appended (row 64 of the PV output then holds the softmax
    denominator).
  - scores^T[k, q] = K^T.T @ Q^T per (b,h); softmax without max subtraction
    (scores are O(1) after LN; exp cannot overflow); exp on ACT directly from
    PSUM; PV: out^T[d|den, q] accumulated over k chunks; PE-transpose back to
    natural [q, d] and multiply by 1/den.
"""

import os
import numpy as np
import ml_dtypes

import concourse.bacc as bacc
import concourse.mybir as mybir
import concourse.tile as tile
from concourse.bass_utils import run_bass_kernel_spmd

F32 = mybir.dt.float32
BF16 = mybir.dt.bfloat16
AF = mybir.ActivationFunctionType
ALU = mybir.AluOpType

S = 2048
B = 2
DIM = 1024
NHEAD = 16
HEAD = 64
EPS = 1e-5
NCORES = 8
OC = DIM // NCORES          # 128 output channels per core
HPC = OC // HEAD            # 2 heads per core
T = S * B                   # 4096 tokens (t = b*S + s)
TCH = 512                   # token chunk (matmul moving free dim)
NT = T // TCH               # 8 token chunks
NCC = DIM // 128            # 8 contraction chunks

LAST_RESULT = None


def _emit(tc, aps, flags):
    from contextlib import ExitStack

    nc = tc.nc
    names = ("q", "k", "v")

    stack = ExitStack()
    consts = stack.enter_context(tc.tile_pool(name="consts", bufs=1))
    # stationary weights W^T as [128, 8, 128] (partition=c within chunk)
    wt_sb = {}
    for n in names:
        t = consts.tile([128, NCC, OC], BF16, tag=f"wt_{n}")
        nc.sync.dma_start(out=t, in_=aps[f"wt_{n}"].rearrange("(a p) m -> p a m", p=128))
        wt_sb[n] = t
    bones32 = consts.tile([128, OC], F32, tag="bones32")
    nc.sync.dma_start(out=bones32, in_=aps["blockones_f32"])
    bones16 = consts.tile([128, OC], BF16, tag="bones16")
    nc.sync.dma_start(out=bones16, in_=aps["blockones_bf16"])
    id16 = consts.tile([128, 128], BF16, tag="id16")
    nc.sync.dma_start(out=id16, in_=aps["identity_bf16"])
    id32 = consts.tile([128, 128], F32, tag="id32")
    nc.sync.dma_start(out=id32, in_=aps["identity_f32"])
    eps_q = consts.tile([128, 1], F32, tag="eps_q")
    nc.vector.memset(eps_q, float(HEAD * EPS))
    eps_kv = consts.tile([128, 1], F32, tag="eps_kv")
    nc.vector.memset(eps_kv, float(EPS))
    extra = {}
    for n in names:
        if flags[f"bias_{n}"]:
            t = consts.tile([128, 1], F32, tag=f"bcol_{n}")
            nc.sync.dma_start(out=t, in_=aps[f"bcol_{n}"])
            extra[f"bcol_{n}"] = t
        if flags[f"gb_{n}"]:
            tg = consts.tile([128, 1], F32, tag=f"gcol_{n}")
            nc.sync.dma_start(out=tg, in_=aps[f"gcol_{n}"])
            tb = consts.tile([128, 1], F32, tag=f"betacol_{n}")
            nc.sync.dma_start(out=tb, in_=aps[f"betacol_{n}"])
            extra[f"gcol_{n}"] = tg
            extra[f"betacol_{n}"] = tb

    ln_pool = stack.enter_context(tc.tile_pool(name="ln", bufs=1))
    ln_sb = {
        n: ln_pool.tile([128, T], BF16, tag=f"ln_{n}", name=f"ln_{n}") for n in names
    }

    # ---------------- phase 1: projections + per-head LN ----------------
    # Software-pipelined: the projection matmul stream runs ahead; each
    # chunk's stats matmuls (which depend on DVE/ACT work) are emitted with
    # a lag of PIPE chunk-slots so the PE FIFO never stalls on them.
    PIPE = 2

    with (
        tc.tile_pool(name="xload", bufs=8) as xload,
        tc.tile_pool(name="p1tmp", bufs=4) as p1tmp,
        tc.tile_pool(name="ps_y", bufs=4, space="PSUM") as ps_y_pool,
        tc.tile_pool(name="ps_stat", bufs=4, space="PSUM") as ps_stat_pool,
    ):
        units = [(tch, n) for tch in range(NT) for n in names]
        state = {}

        def emit_proj(tch, n):
            tsl = slice(tch * TCH, (tch + 1) * TCH)
            ps_y = ps_y_pool.tile([128, TCH], F32, tag="ps_y", name="ps_y")
            for c in range(NCC):
                xt = xload.tile([128, TCH], BF16, tag="xt", name="xt")
                nc.sync.dma_start(
                    out=xt, in_=aps[f"xt_{n}"][c * 128:(c + 1) * 128, tsl]
                )
                nc.tensor.matmul(
                    ps_y, lhsT=wt_sb[n][:, c, :], rhs=xt,
                    start=(c == 0), stop=(c == NCC - 1),
                )
            # DVE/ACT chain feeding the (later-emitted) stats matmuls
            yt32 = p1tmp.tile([128, TCH], F32, tag="yt32", name="yt32")
            if flags[f"bias_{n}"]:
                nc.vector.tensor_scalar(
                    out=yt32, in0=ps_y, scalar1=extra[f"bcol_{n}"],
                    scalar2=None, op0=ALU.add,
                )
            else:
                nc.scalar.copy(out=yt32, in_=ps_y)
            state[(tch, n)] = yt32

        def emit_stats(tch, n):
            tsl = slice(tch * TCH, (tch + 1) * TCH)
            yt32 = state.pop((tch, n))
            ps_mu = ps_stat_pool.tile([128, TCH], F32, tag="stat", name="ps_mu")
            nc.tensor.matmul(ps_mu, lhsT=bones32, rhs=yt32, start=True, stop=True)
            t_c = p1tmp.tile([128, TCH], F32, tag="t_c", name="t_c")
            nc.vector.tensor_sub(t_c, yt32, ps_mu)
            sq = p1tmp.tile([128, TCH], BF16, tag="sq", name="sq")
            nc.scalar.square(sq, t_c)
            state[(tch, n, "var")] = (t_c, sq, tsl)

        def emit_norm(tch, n):
            t_c, sq, tsl = state.pop((tch, n, "var"))
            ps_var = ps_stat_pool.tile([128, TCH], F32, tag="stat", name="ps_var")
            nc.tensor.matmul(ps_var, lhsT=bones16, rhs=sq, start=True, stop=True)
            std = p1tmp.tile([128, TCH], F32, tag="std", name="std")
            if n == "q":
                # std8 = sqrt(HEAD*var + HEAD*eps) = sqrt(HEAD)*sqrt(var+eps)
                nc.scalar.activation(std, ps_var, AF.Sqrt, bias=eps_q, scale=float(HEAD))
            else:
                nc.scalar.activation(std, ps_var, AF.Sqrt, bias=eps_kv, scale=1.0)
            s_t = p1tmp.tile([128, TCH], F32, tag="s_t", name="s_t")
            nc.vector.reciprocal(s_t, std)
            if flags[f"gb_{n}"]:
                lnf = p1tmp.tile([128, TCH], F32, tag="lnf", name="lnf")
                nc.vector.tensor_mul(lnf, t_c, s_t)
                nc.vector.tensor_scalar(
                    out=ln_sb[n][:, tsl], in0=lnf,
                    scalar1=extra[f"gcol_{n}"], scalar2=extra[f"betacol_{n}"],
                    op0=ALU.mult, op1=ALU.add,
                )
            else:
                nc.vector.tensor_mul(ln_sb[n][:, tsl], t_c, s_t)

        # interleave: proj(i) ... stats(i - PIPE) ... norm(i - PIPE - 1)
        for i, (tch, n) in enumerate(units):
            emit_proj(tch, n)
            if i >= PIPE:
                emit_stats(*units[i - PIPE])
            if i >= PIPE + 1:
                emit_norm(*units[i - PIPE - 1])
        for i in range(len(units) - PIPE, len(units)):
            emit_stats(*units[i])
        for i in range(len(units) - PIPE - 1, len(units)):
            emit_norm(*units[i])

    # ---------------- phases 2+3: per-(b, qc) attention, heads row-paired --
    QCH = 512
    NQ = S // QCH            # 4 q chunks per (b, h)
    NKT = S // 128           # 16 k tiles per (b, h)

    with (
        tc.tile_pool(name="vnat", bufs=4) as vnat_pool,
        tc.tile_pool(name="attn", bufs=2) as attn_pool,
        tc.tile_pool(name="p3tmp", bufs=4) as p3tmp,
        tc.tile_pool(name="ostage", bufs=4) as ostage_pool,
        tc.tile_pool(name="ps_wave", bufs=2, space="PSUM") as ps_wave_pool,
        tc.tile_pool(name="ps_o", bufs=2, space="PSUM") as ps_o_pool,
        tc.tile_pool(name="ps_tr", bufs=2, space="PSUM") as ps_tr_pool,
    ):
        vnats_by_b = {}
        ostages_by_b = {}

        def emit_vnat_ostage(b):
            t0 = b * S
            vs, osts = [], []
            for hl in range(HPC):
                dsl = slice(HEAD * hl, HEAD * (hl + 1))
                vnat = vnat_pool.tile(
                    [128, NKT, HEAD + 1], BF16, tag="vnat", name=f"vnat{b}{hl}"
                )
                nc.vector.memset(vnat[:, :, HEAD:HEAD + 1], 1.0)
                for kt in range(NKT):
                    ps_tr = ps_tr_pool.tile([128, HEAD], BF16, tag="tr", name="ps_tr")
                    nc.tensor.transpose(
                        ps_tr,
                        ln_sb["v"][dsl, t0 + kt * 128: t0 + (kt + 1) * 128],
                        id16[dsl, dsl],
                    )
                    nc.vector.tensor_copy(out=vnat[:, kt, 0:HEAD], in_=ps_tr)
                vs.append(vnat)
                osts.append(
                    ostage_pool.tile(
                        [128, S // 128, HEAD], F32, tag="ostage",
                        name=f"ostage{b}{hl}",
                    )
                )
            vnats_by_b[b] = vs
            ostages_by_b[b] = osts

        def emit_pv(pu, kt):
            for hl in range(HPC):
                nc.tensor.matmul(
                    pu["ps_o"][hl],
                    lhsT=vnats_by_b[pu["b"]][hl][:, kt, :],
                    rhs=pu["at_q"][:, kt, hl, :],
                    start=(kt == 0), stop=(kt == NKT - 1),
                )

        def emit_finish(pu):
            b, qc = pu["b"], pu["qc"]
            for hl in range(HPC):
                # oT rows 0..63 = out^T values; row 64 = 1/den (reciprocal of
                # the denominator row, computed once per 512 queries). The
                # transpose then carries inv_den into column 64 of each tile.
                oT = p3tmp.tile([HEAD + 1, QCH], F32, tag="oT", name="oT")
                nc.vector.tensor_copy(out=oT[:HEAD, :], in_=pu["ps_o"][hl][:HEAD, :])
                nc.vector.reciprocal(
                    oT[HEAD:HEAD + 1, :], pu["ps_o"][hl][HEAD:HEAD + 1, :]
                )
                for sub in range(QCH // 128):
                    ps_tro = ps_tr_pool.tile(
                        [128, HEAD + 1], F32, tag="tr", name="ps_tro"
                    )
                    nc.tensor.transpose(
                        ps_tro, oT[:, sub * 128:(sub + 1) * 128],
                        id32[:HEAD + 1, :HEAD + 1],
                    )
                    nc.vector.tensor_scalar(
                        out=ostages_by_b[b][hl][:, qc * (QCH // 128) + sub, :],
                        in0=ps_tro[:, 0:HEAD],
                        scalar1=ps_tro[:, HEAD:HEAD + 1],
                        scalar2=None, op0=ALU.mult,
                    )
            if qc == NQ - 1:
                for hl in range(HPC):
                    dst = aps["out"][:, b, HEAD * hl: HEAD * (hl + 1)]
                    dst = dst.rearrange("(n p) c -> p n c", p=128)
                    nc.sync.dma_start(out=dst, in_=ostages_by_b[b][hl])

        att_units = [(b, qc) for b in range(B) for qc in range(NQ)]
        prev = None
        for b, qc in att_units:
            if qc == 0:
                emit_vnat_ostage(b)
            t0 = b * S
            at_q = attn_pool.tile(
                [128, NKT, HPC, QCH], BF16, tag="at", name="at_q"
            )
            if prev is not None:
                prev["ps_o"] = [
                    ps_o_pool.tile([HEAD + 1, QCH], F32, tag="ps_o", name="ps_o")
                    for _ in range(HPC)
                ]
            for kt in range(NKT):
                ps_wave = ps_wave_pool.tile(
                    [128, HPC, QCH], F32, tag="wave", name="ps_wave"
                )
                for hl in range(HPC):
                    dsl = slice(HEAD * hl, HEAD * (hl + 1))
                    nc.tensor.matmul(
                        ps_wave[:, hl, :],
                        lhsT=ln_sb["k"][dsl, t0 + kt * 128: t0 + (kt + 1) * 128],
                        rhs=ln_sb["q"][dsl, t0 + qc * QCH: t0 + (qc + 1) * QCH],
                        start=True, stop=True,
                    )
                nc.scalar.activation(at_q[:, kt], ps_wave, AF.Exp)
                if prev is not None:
                    emit_pv(prev, kt)
            if prev is not None:
                emit_finish(prev)
            prev = {"b": b, "qc": qc, "at_q": at_q}
        # drain the last unit
        prev["ps_o"] = [
            ps_o_pool.tile([HEAD + 1, QCH], F32, tag="ps_o", name="ps_o")
            for _ in range(HPC)
        ]
        for kt in range(NKT):
            emit_pv(prev, kt)
        emit_finish(prev)

    stack.close()


def _build(flags_key, flags, input_specs):
    nc = bacc.Bacc("TRN2", target_bir_lowering=False, debug=False)
    aps = {}
    for name, shape, dt in input_specs:
        aps[name] = nc.dram_tensor(name, list(shape), dt, kind="ExternalInput").ap()
    aps["out"] = nc.dram_tensor("out", [S, B, OC], F32, kind="ExternalOutput").ap()
    with tile.TileContext(nc) as tc:
        _emit(tc, aps, flags)
    nc.compile()
    return nc


_CACHE = {}


def kernel(**inputs):
    global LAST_RESULT
    bf16 = ml_dtypes.bfloat16
    f32 = np.float32

    Q, K, V = (np.asarray(inputs[n], f32) for n in ("Q", "K", "V"))
    W = {n: np.asarray(inputs["W" + n.upper()], f32) for n in ("q", "k", "v")}
    bias = {n: np.asarray(inputs["b" + n.upper()], f32) for n in ("q", "k", "v")}
    g = {n: np.asarray(inputs["g" + n.upper()], f32) for n in ("q", "k", "v")}
    beta = {n: np.asarray(inputs["beta" + n.upper()], f32) for n in ("q", "k", "v")}

    # X^T [c, t] with t = b*S + s
    xt = {
        "q": np.ascontiguousarray(Q.transpose(2, 1, 0).reshape(DIM, T)).astype(bf16),
        "k": np.ascontiguousarray(K.transpose(2, 1, 0).reshape(DIM, T)).astype(bf16),
        "v": np.ascontiguousarray(V.transpose(2, 1, 0).reshape(DIM, T)).astype(bf16),
    }
    blockones = np.kron(np.eye(2, dtype=f32), np.ones((HEAD, HEAD), f32)) / HEAD
    ident = np.eye(128, dtype=f32)

    flags = {}
    for n in ("q", "k", "v"):
        flags[f"bias_{n}"] = bool(np.any(bias[n] != 0.0))
        flags[f"gb_{n}"] = bool(np.any(g[n] != 1.0) or np.any(beta[n] != 0.0))
    flags_key = tuple(sorted(flags.items()))

    # per-core input maps
    in_maps = []
    shared = {
        "xt_q": xt["q"], "xt_k": xt["k"], "xt_v": xt["v"],
        "blockones_f32": blockones,
        "blockones_bf16": blockones.astype(bf16),
        "identity_f32": ident,
        "identity_bf16": ident.astype(bf16),
    }
    for p in range(NCORES):
        sl = slice(OC * p, OC * (p + 1))
        m = dict(shared)
        for n in ("q", "k", "v"):
            m[f"wt_{n}"] = np.ascontiguousarray(W[n][sl].T).astype(bf16)
            if flags[f"bias_{n}"]:
                m[f"bcol_{n}"] = np.ascontiguousarray(bias[n][sl]).reshape(128, 1)
            if flags[f"gb_{n}"]:
                m[f"gcol_{n}"] = np.tile(g[n], HPC).astype(f32).reshape(128, 1)
                bcol = np.tile(beta[n], HPC).astype(f32)
                if n == "q":
                    bcol = (bcol / np.sqrt(HEAD)).astype(f32)
                m[f"betacol_{n}"] = bcol.reshape(128, 1)
        in_maps.append(m)

    if flags_key not in _CACHE:
        input_specs = []
        for name, arr in in_maps[0].items():
            dt = BF16 if arr.dtype == bf16 else F32
            input_specs.append((name, arr.shape, dt))
        _CACHE[flags_key] = _build(flags_key, flags, input_specs)
    nc = _CACHE[flags_key]

    trace = bool(os.environ.get("KERNEL_TRACE"))
    tmpdir = os.environ.get("KERNEL_TRACE_DIR") or None
    res = run_bass_kernel_spmd(
        nc, in_maps, core_ids=list(range(NCORES)), trace=trace, tmpdir=tmpdir
    )
    LAST_RESULT = res
    out = np.concatenate(
        [np.asarray(res.results[p]["out"], f32) for p in range(NCORES)], axis=2
    )
    return out


# revision 22
# speedup vs baseline: 1.8286x; 1.4436x over previous
"""Trainium2 Bass kernel for nn_CrossAttention (per-head-LN cross attention).

Sharding: 16 heads / 8 cores -> 2 heads per core, both batch elements on every
core (attention is embarrassingly parallel over (B, H)). Each core computes its
128 output channels [128p, 128p+128) of the final [S, B, 1024] output.

Device algorithm (per core, all matmuls bf16 with f32 PSUM accumulation):
  - Projections computed transposed: Y^T[o, t] (o = core's 128 channels,
    t = b*S + s), via stationary W^T chunks against streamed X^T tiles.
  - Per-head LayerNorm with matmul-broadcast stats: mu_bc = blockones.T @ Y
    (f32), var_bc = blockones.T @ (Y - mu)^2 (bf16), rstd via ACT Sqrt + DVE
    reciprocal. The 1/sqrt(head) score scale is folded into Q's Sqrt scale.
  - V transposed back to natural [k, d] per (b,h) via PE transposes, with a
    ones column appended (row 64 of the PV output then holds the softmax
    denominator).
  - scores^T[k, q] = K^T.T @ Q^T per (b,h); softmax without max subtraction
    (scores are O(1) after LN; exp cannot overflow); exp on ACT directly from
    PSUM; PV: out^T[d|den, q] accumulated over k chunks; PE-transpose back to
    natural [q, d] and multiply by 1/den.
"""

import os
import numpy as np
import ml_dtypes

import concourse.bacc as bacc
import concourse.mybir as mybir
import concourse.tile as tile
from concourse.bass_utils import run_bass_kernel_spmd

F32 = mybir.dt.float32
BF16 = mybir.dt.bfloat16
AF = mybir.ActivationFunctionType
ALU = mybir.AluOpType

S = 2048
B = 2
DIM = 1024
NHEAD = 16
HEAD = 64
EPS = 1e-5
NCORES = 8
OC = DIM // NCORES          # 128 output channels per core
HPC = OC // HEAD            # 2 heads per core
T = S * B                   # 4096 tokens (t = b*S + s)
TCH = 512                   # token chunk (matmul moving free dim)
NT = T // TCH               # 8 token chunks
NCC = DIM // 128            # 8 contraction chunks

LAST_RESULT = None


def _emit(tc, aps, flags):
    from contextlib import ExitStack

    nc = tc.nc
    names = ("q", "k", "v")

    stack = ExitStack()
    consts = stack.enter_context(tc.tile_pool(name="consts", bufs=1))
    # stationary weights W^T as [128, 8, 128] (partition=c within chunk)
    wt_sb = {}
    for n in names:
        t = consts.tile([128, NCC, OC], BF16, tag=f"wt_{n}")
        nc.sync.dma_start(out=t, in_=aps[f"wt_{n}"].rearrange("(a p) m -> p a m", p=128))
        wt_sb[n] = t
    bones32 = consts.tile([128, OC], F32, tag="bones32")
    nc.sync.dma_start(out=bones32, in_=aps["blockones_f32"])
    bones16 = consts.tile([128, OC], BF16, tag="bones16")
    nc.sync.dma_start(out=bones16, in_=aps["blockones_bf16"])
    id16 = consts.tile([128, 128], BF16, tag="id16")
    nc.sync.dma_start(out=id16, in_=aps["identity_bf16"])
    id32 = consts.tile([128, 128], F32, tag="id32")
    nc.sync.dma_start(out=id32, in_=aps["identity_f32"])
    eps_q = consts.tile([128, 1], F32, tag="eps_q")
    nc.vector.memset(eps_q, float(HEAD * EPS))
    eps_kv = consts.tile([128, 1], F32, tag="eps_kv")
    nc.vector.memset(eps_kv, float(EPS))
    extra = {}
    for n in names:
        if flags[f"bias_{n}"]:
            t = consts.tile([128, 1], F32, tag=f"bcol_{n}")
            nc.sync.dma_start(out=t, in_=aps[f"bcol_{n}"])
            extra[f"bcol_{n}"] = t
        if flags[f"gb_{n}"]:
            tg = consts.tile([128, 1], F32, tag=f"gcol_{n}")
            nc.sync.dma_start(out=tg, in_=aps[f"gcol_{n}"])
            tb = consts.tile([128, 1], F32, tag=f"betacol_{n}")
            nc.sync.dma_start(out=tb, in_=aps[f"betacol_{n}"])
            extra[f"gcol_{n}"] = tg
            extra[f"betacol_{n}"] = tb

    ln_pool = stack.enter_context(tc.tile_pool(name="ln", bufs=1))
    ln_sb = {
        n: ln_pool.tile([128, T], BF16, tag=f"ln_{n}", name=f"ln_{n}") for n in names
    }

    # ---------------- phase 1: projections + per-head LN ----------------
    # Software-pipelined: the projection matmul stream runs ahead; each
    # chunk's stats matmuls (which depend on DVE/ACT work) are emitted with
    # a lag of PIPE chunk-slots so the PE FIFO never stalls on them.
    PIPE = 2

    with (
        tc.tile_pool(name="xload", bufs=16) as xload,
        tc.tile_pool(name="p1tmp", bufs=4) as p1tmp,
        tc.tile_pool(name="ps_y", bufs=4, space="PSUM") as ps_y_pool,
        tc.tile_pool(name="ps_stat", bufs=4, space="PSUM") as ps_stat_pool,
    ):
        # pair-major order: both halves of a 1024-token DMA load are consumed
        # by consecutive units of the same tensor
        units = [
            (tp * 2 + half, n) for tp in range(NT // 2) for n in names
            for half in range(2)
        ]
        state = {}
        xt2_cache = {}

        def emit_proj(tch, n):
            tp, half = tch // 2, tch % 2
            if half == 0:
                tiles = []
                psl = slice(tp * 2 * TCH, (tp * 2 + 2) * TCH)
                for c in range(NCC):
                    xt = xload.tile([128, 2 * TCH], BF16, tag="xt", name="xt")
                    nc.sync.dma_start(
                        out=xt, in_=aps[f"xt_{n}"][c * 128:(c + 1) * 128, psl]
                    )
                    tiles.append(xt)
                xt2_cache[n] = tiles
            tsl = slice(tch * TCH, (tch + 1) * TCH)
            ps_y = ps_y_pool.tile([128, TCH], F32, tag="ps_y", name="ps_y")
            for c in range(NCC):
                nc.tensor.matmul(
                    ps_y, lhsT=wt_sb[n][:, c, :],
                    rhs=xt2_cache[n][c][:, half * TCH:(half + 1) * TCH],
                    start=(c == 0), stop=(c == NCC - 1),
                )
            # DVE/ACT chain feeding the (later-emitted) stats matmuls
            yt32 = p1tmp.tile([128, TCH], F32, tag="yt32", name="yt32")
            if flags[f"bias_{n}"]:
                nc.vector.tensor_scalar(
                    out=yt32, in0=ps_y, scalar1=extra[f"bcol_{n}"],
                    scalar2=None, op0=ALU.add,
                )
            else:
                nc.scalar.copy(out=yt32, in_=ps_y)
            state[(tch, n)] = yt32

        def emit_stats(tch, n):
            tsl = slice(tch * TCH, (tch + 1) * TCH)
            yt32 = state.pop((tch, n))
            ps_mu = ps_stat_pool.tile([128, TCH], F32, tag="stat", name="ps_mu")
            nc.tensor.matmul(ps_mu, lhsT=bones32, rhs=yt32, start=True, stop=True)
            t_c = p1tmp.tile([128, TCH], F32, tag="t_c", name="t_c")
            nc.vector.tensor_sub(t_c, yt32, ps_mu)
            sq = p1tmp.tile([128, TCH], BF16, tag="sq", name="sq")
            nc.scalar.square(sq, t_c)
            state[(tch, n, "var")] = (t_c, sq, tsl)

        def emit_norm(tch, n):
            t_c, sq, tsl = state.pop((tch, n, "var"))
            ps_var = ps_stat_pool.tile([128, TCH], F32, tag="stat", name="ps_var")
            nc.tensor.matmul(ps_var, lhsT=bones16, rhs=sq, start=True, stop=True)
            std = p1tmp.tile([128, TCH], F32, tag="std", name="std")
            if n == "q":
                # std8 = sqrt(HEAD*var + HEAD*eps) = sqrt(HEAD)*sqrt(var+eps)
                nc.scalar.activation(std, ps_var, AF.Sqrt, bias=eps_q, scale=float(HEAD))
            else:
                nc.scalar.activation(std, ps_var, AF.Sqrt, bias=eps_kv, scale=1.0)
            s_t = p1tmp.tile([128, TCH], F32, tag="s_t", name="s_t")
            nc.vector.reciprocal_approx_fast(s_t, std)
            if flags[f"gb_{n}"]:
                lnf = p1tmp.tile([128, TCH], F32, tag="lnf", name="lnf")
                nc.vector.tensor_mul(lnf, t_c, s_t)
                nc.vector.tensor_scalar(
                    out=ln_sb[n][:, tsl], in0=lnf,
                    scalar1=extra[f"gcol_{n}"], scalar2=extra[f"betacol_{n}"],
                    op0=ALU.mult, op1=ALU.add,
                )
            else:
                nc.vector.tensor_mul(ln_sb[n][:, tsl], t_c, s_t)

        # interleave: proj(i) ... stats(i - PIPE) ... norm(i - PIPE - 1)
        for i, (tch, n) in enumerate(units):
            emit_proj(tch, n)
            if i >= PIPE:
                emit_stats(*units[i - PIPE])
            if i >= PIPE + 1:
                emit_norm(*units[i - PIPE - 1])
        for i in range(len(units) - PIPE, len(units)):
            emit_stats(*units[i])
        for i in range(len(units) - PIPE - 1, len(units)):
            emit_norm(*units[i])

    # ---------------- phases 2+3: per-(b, qc) attention, heads row-paired --
    QCH = 512
    NQ = S // QCH            # 4 q chunks per (b, h)
    NKT = S // 128           # 16 k tiles per (b, h)

    with (
        tc.tile_pool(name="vnat", bufs=4) as vnat_pool,
        tc.tile_pool(name="attn", bufs=2) as attn_pool,
        tc.tile_pool(name="p3tmp", bufs=4) as p3tmp,
        tc.tile_pool(name="ostage", bufs=4) as ostage_pool,
        tc.tile_pool(name="ps_wave", bufs=2, space="PSUM") as ps_wave_pool,
        tc.tile_pool(name="ps_o", bufs=2, space="PSUM") as ps_o_pool,
        tc.tile_pool(name="ps_tr", bufs=2, space="PSUM") as ps_tr_pool,
    ):
        vnats_by_b = {}
        ostages_by_b = {}

        def emit_vnat_ostage(b):
            t0 = b * S
            vs, osts = [], []
            for hl in range(HPC):
                dsl = slice(HEAD * hl, HEAD * (hl + 1))
                vnat = vnat_pool.tile(
                    [128, NKT, HEAD + 1], BF16, tag="vnat", name=f"vnat{b}{hl}"
                )
                nc.vector.memset(vnat[:, :, HEAD:HEAD + 1], 1.0)
                for kt in range(NKT):
                    ps_tr = ps_tr_pool.tile([128, HEAD], BF16, tag="tr", name="ps_tr")
                    nc.tensor.transpose(
                        ps_tr,
                        ln_sb["v"][dsl, t0 + kt * 128: t0 + (kt + 1) * 128],
                        id16[dsl, dsl],
                    )
                    nc.vector.tensor_copy(out=vnat[:, kt, 0:HEAD], in_=ps_tr)
                vs.append(vnat)
                osts.append(
                    ostage_pool.tile(
                        [128, S // 128, HEAD], F32, tag="ostage",
                        name=f"ostage{b}{hl}",
                    )
                )
            vnats_by_b[b] = vs
            ostages_by_b[b] = osts

        def emit_pv(pu, kt):
            for hl in range(HPC):
                nc.tensor.matmul(
                    pu["ps_o"][hl],
                    lhsT=vnats_by_b[pu["b"]][hl][:, kt, :],
                    rhs=pu["at_q"][:, kt, hl, :],
                    start=(kt == 0), stop=(kt == NKT - 1),
                )

        def emit_finish(pu):
            b, qc = pu["b"], pu["qc"]
            for hl in range(HPC):
                # oT rows 0..63 = out^T values; row 64 = 1/den (reciprocal of
                # the denominator row, computed once per 512 queries). The
                # transpose then carries inv_den into column 64 of each tile.
                oT = p3tmp.tile([HEAD + 1, QCH], F32, tag="oT", name="oT")
                nc.vector.tensor_copy(out=oT[:HEAD, :], in_=pu["ps_o"][hl][:HEAD, :])
                # reciprocal_approx_fast requires base_partition 0 on both
                # operands (HW uop quirk) — stage the den row through base 0.
                den = p3tmp.tile([1, QCH], F32, tag="den", name="den")
                nc.vector.tensor_copy(out=den, in_=pu["ps_o"][hl][HEAD:HEAD + 1, :])
                inv = p3tmp.tile([1, QCH], F32, tag="inv", name="inv")
                nc.vector.reciprocal_approx_fast(inv, den)
                nc.vector.tensor_copy(out=oT[HEAD:HEAD + 1, :], in_=inv)
                for sub in range(QCH // 128):
                    ps_tro = ps_tr_pool.tile(
                        [128, HEAD + 1], F32, tag="tr", name="ps_tro"
                    )
                    nc.tensor.transpose(
                        ps_tro, oT[:, sub * 128:(sub + 1) * 128],
                        id32[:HEAD + 1, :HEAD + 1],
                    )
                    nc.vector.tensor_scalar(
                        out=ostages_by_b[b][hl][:, qc * (QCH // 128) + sub, :],
                        in0=ps_tro[:, 0:HEAD],
                        scalar1=ps_tro[:, HEAD:HEAD + 1],
                        scalar2=None, op0=ALU.mult,
                    )
            if qc == NQ - 1:
                for hl in range(HPC):
                    dst = aps["out"][:, b, HEAD * hl: HEAD * (hl + 1)]
                    dst = dst.rearrange("(n p) c -> p n c", p=128)
                    nc.sync.dma_start(out=dst, in_=ostages_by_b[b][hl])

        att_units = [(b, qc) for b in range(B) for qc in range(NQ)]
        prev = None
        for b, qc in att_units:
            if qc == 0:
                emit_vnat_ostage(b)
            t0 = b * S
            at_q = attn_pool.tile(
                [128, NKT, HPC, QCH], BF16, tag="at", name="at_q"
            )
            if prev is not None:
                prev["ps_o"] = [
                    ps_o_pool.tile([HEAD + 1, QCH], F32, tag="ps_o", name="ps_o")
                    for _ in range(HPC)
                ]
            for kt in range(NKT):
                ps_wave = ps_wave_pool.tile(
                    [128, HPC, QCH], F32, tag="wave", name="ps_wave"
                )
                for hl in range(HPC):
                    dsl = slice(HEAD * hl, HEAD * (hl + 1))
                    nc.tensor.matmul(
                        ps_wave[:, hl, :],
                        lhsT=ln_sb["k"][dsl, t0 + kt * 128: t0 + (kt + 1) * 128],
                        rhs=ln_sb["q"][dsl, t0 + qc * QCH: t0 + (qc + 1) * QCH],
                        start=True, stop=True,
                    )
                nc.scalar.activation(at_q[:, kt], ps_wave, AF.Exp)
                if prev is not None:
                    emit_pv(prev, kt)
            if prev is not None:
                emit_finish(prev)
            prev = {"b": b, "qc": qc, "at_q": at_q}
        # drain the last unit
        prev["ps_o"] = [
            ps_o_pool.tile([HEAD + 1, QCH], F32, tag="ps_o", name="ps_o")
            for _ in range(HPC)
        ]
        for kt in range(NKT):
            emit_pv(prev, kt)
        emit_finish(prev)

    stack.close()


def _build(flags_key, flags, input_specs):
    nc = bacc.Bacc("TRN2", target_bir_lowering=False, debug=False)
    aps = {}
    for name, shape, dt in input_specs:
        aps[name] = nc.dram_tensor(name, list(shape), dt, kind="ExternalInput").ap()
    aps["out"] = nc.dram_tensor("out", [S, B, OC], F32, kind="ExternalOutput").ap()
    with tile.TileContext(nc) as tc:
        _emit(tc, aps, flags)
    nc.compile()
    return nc


_CACHE = {}


def kernel(**inputs):
    global LAST_RESULT
    bf16 = ml_dtypes.bfloat16
    f32 = np.float32

    Q, K, V = (np.asarray(inputs[n], f32) for n in ("Q", "K", "V"))
    W = {n: np.asarray(inputs["W" + n.upper()], f32) for n in ("q", "k", "v")}
    bias = {n: np.asarray(inputs["b" + n.upper()], f32) for n in ("q", "k", "v")}
    g = {n: np.asarray(inputs["g" + n.upper()], f32) for n in ("q", "k", "v")}
    beta = {n: np.asarray(inputs["beta" + n.upper()], f32) for n in ("q", "k", "v")}

    # X^T [c, t] with t = b*S + s
    xt = {
        "q": np.ascontiguousarray(Q.transpose(2, 1, 0).reshape(DIM, T)).astype(bf16),
        "k": np.ascontiguousarray(K.transpose(2, 1, 0).reshape(DIM, T)).astype(bf16),
        "v": np.ascontiguousarray(V.transpose(2, 1, 0).reshape(DIM, T)).astype(bf16),
    }
    blockones = np.kron(np.eye(2, dtype=f32), np.ones((HEAD, HEAD), f32)) / HEAD
    ident = np.eye(128, dtype=f32)

    flags = {}
    for n in ("q", "k", "v"):
        flags[f"bias_{n}"] = bool(np.any(bias[n] != 0.0))
        flags[f"gb_{n}"] = bool(np.any(g[n] != 1.0) or np.any(beta[n] != 0.0))
    flags_key = tuple(sorted(flags.items()))

    # per-core input maps
    in_maps = []
    shared = {
        "xt_q": xt["q"], "xt_k": xt["k"], "xt_v": xt["v"],
        "blockones_f32": blockones,
        "blockones_bf16": blockones.astype(bf16),
        "identity_f32": ident,
        "identity_bf16": ident.astype(bf16),
    }
    for p in range(NCORES):
        sl = slice(OC * p, OC * (p + 1))
        m = dict(shared)
        for n in ("q", "k", "v"):
            m[f"wt_{n}"] = np.ascontiguousarray(W[n][sl].T).astype(bf16)
            if flags[f"bias_{n}"]:
                m[f"bcol_{n}"] = np.ascontiguousarray(bias[n][sl]).reshape(128, 1)
            if flags[f"gb_{n}"]:
                m[f"gcol_{n}"] = np.tile(g[n], HPC).astype(f32).reshape(128, 1)
                bcol = np.tile(beta[n], HPC).astype(f32)
                if n == "q":
                    bcol = (bcol / np.sqrt(HEAD)).astype(f32)
                m[f"betacol_{n}"] = bcol.reshape(128, 1)
        in_maps.append(m)

    if flags_key not in _CACHE:
        input_specs = []
        for name, arr in in_maps[0].items():
            dt = BF16 if arr.dtype == bf16 else F32
            input_specs.append((name, arr.shape, dt))
        _CACHE[flags_key] = _build(flags_key, flags, input_specs)
    nc = _CACHE[flags_key]

    trace = bool(os.environ.get("KERNEL_TRACE"))
    tmpdir = os.environ.get("KERNEL_TRACE_DIR") or None
    res = run_bass_kernel_spmd(
        nc, in_maps, core_ids=list(range(NCORES)), trace=trace, tmpdir=tmpdir
    )
    LAST_RESULT = res
    out = np.concatenate(
        [np.asarray(res.results[p]["out"], f32) for p in range(NCORES)], axis=2
    )
    return out
